# revision 111
# baseline (speedup 1.0000x reference)
"""Abbott STDP step kernel for 8 Trainium2 NeuronCores.

Math (per reference):
  dW_pot[b,e,o] = Xpost[b,o]   * (sum_d xbar_pre[d,b,e]*dmap[d,e,o]) * A_p[e,o]
  dW_dep[b,e,o] = xbar_post[b,o]* (sum_d Xd[d,b,e]      *dmap[d,e,o]) * A_d[e,o]
  W_new = clip(W + dW_pot - dW_dep, 0, 1)
  xbar_pre_new  = 0.95*xbar_pre  + 0.05*Xd      (host; trivially small)
  xbar_post_new = 0.90*xbar_post + 0.10*Xpost   (host; trivially small)

Sharding: presynaptic axis e (axis 1 of dmap/W/A, axis 2 of xbar_pre/Xd)
split into 8 slabs of 256; no cross-device reduction (d is the only
contracted axis).

Per-core compute:
  - d-contraction on the TensorEngine: 8 PSUM-accumulated matmuls per
    (b, pot/dep) output tile with diag(xbar[d,b,.]) stationary, in bf16
    (1 cycle/row; fp32 PSUM accumulation).  The diagonals are built once
    on DVE from a single packed host constant [ident | xps | xds].
  - The per-column masks and amplitudes are folded on the host into
    ApX[b]=A_p*Xpost[b]*65535, AdX[b]=A_d*xbar_post[b]*65535 (bf16), so
    the on-chip combine is just: t1=S_pot*ApX, t2=S_dep*AdX (DVE, PSUM
    source), u=W+t1-t2 (the add reads the u16 W tile directly; DVE
    converts inline, exactly), clip to [0,65535] (DVE fused max/min
    tensor_scalar casting to uint16).
  - W rides in 16-bit fixed point (W*65535 as uint16, exact to 7.6e-6
    since W_new is clipped to [0,1]): halves the W round-trip traffic.
    The u16 buffers wear a float16 facade at the DRAM/PJRT boundary
    (PJRT cannot bind u16); APs are bitcast inside the kernel.
  - dmap streams as quartered [128, 8*512] bf16 DMAs per (e-tile,
    o-chunk) on the SP HWDGE ring with 8-deep prefetch; W rides the
    same ring; ApX/AdX loads and W_new writes ride the ACT ring.
"""

import ml_dtypes
import numpy as np

import concourse.bass as bass
import concourse.mybir as mybir
from concourse.tile import TileContext
from concourse.bass_utils import run_bass_kernel_spmd

D, B, N = 8, 2, 2048
M = 8                  # cores
E = N // M             # 256 — per-core e-slab
OC = 512               # o chunk width
N_ETILES = E // 128    # 2
N_OCHUNKS = N // OC    # 4
ALPHA_P, ALPHA_D = 0.95, 0.9
WMAX = 1.0
NDB = N_ETILES * D * B  # 32 scalar columns per family

f32 = mybir.dt.float32
bf16 = mybir.dt.bfloat16
u16 = mybir.dt.uint16
f16 = mybir.dt.float16


def _split_waits(nc: bass.Bass):
    """Hoist all but one sem wait per instruction onto standalone
    same-engine InstNoOp carriers placed immediately before it.

    The walrus build used here rejects any instruction carrying more
    than one sync wait ("Too many sync wait commands"); engines execute
    in order, so a preceding same-engine wait is equivalent.
    """
    ctr = 0
    for f in nc.m.functions:
        for blk in f.blocks:
            il = blk.instructions
            i = 0
            while i < len(il):
                inst = il[i]
                si = inst.sync_info
                if si is not None and si.on_wait and len(si.on_wait) > 1:
                    waits = list(si.on_wait)
                    inst.sync_info = mybir.SyncInfo(
                        on_wait=[waits[-1]], on_update=list(si.on_update or [])
                    )
                    for w in waits[:-1]:
                        nop = mybir.InstNoOp(name=f"wsplit-{ctr}", ins=[], outs=[])
                        ctr += 1
                        nop.engine = inst.engine
                        nop.sync_info = mybir.SyncInfo(on_wait=[w], on_update=[])
                        il.insert(i, nop)
                        i += 1
                i += 1


def _emit(nc: bass.Bass):
    dmap_s = nc.dram_tensor("dmap_s", [D, E, N], bf16, kind="ExternalInput")
    ApX_s = nc.dram_tensor("ApX_s", [B, E, N], bf16, kind="ExternalInput")
    AdX_s = nc.dram_tensor("AdX_s", [B, E, N], bf16, kind="ExternalInput")
    W_s = nc.dram_tensor("W_s", [B, E, N], f16, kind="ExternalInput")
    setup = nc.dram_tensor("setup", [128, 128 + 2 * NDB], f32, kind="ExternalInput")

    Wout_s = nc.dram_tensor("Wout_s", [B, E, N], f16, kind="ExternalOutput")

    with TileContext(nc) as tc:
        with (
            tc.tile_pool(name="persist", bufs=1) as persist,
            tc.tile_pool(name="stream", bufs=2) as stream,
            tc.tile_pool(name="temps", bufs=4) as temps,
            tc.tile_pool(name="outs", bufs=4) as outs,
            tc.tile_pool(name="psum", bufs=2, space="PSUM") as psum,
        ):
            # ---- one-time setup: one small DMA at the head of the SP ring ----
            setup_t = persist.tile([128, 128 + 2 * NDB], f32, name="setup_t")
            nc.sync.dma_start(out=setup_t, in_=setup[:, :])
            ident_t = setup_t[:, 0:128]

            def xp_col(t, d, b):
                c = 128 + t * 16 + d * 2 + b
                return setup_t[:, c : c + 1]

            def xd_col(t, d, b):
                c = 128 + NDB + t * 16 + d * 2 + b
                return setup_t[:, c : c + 1]

            # diag matrices diag(xbar_pre[d,b,e_tile]), diag(Xd[d,b,e_tile]),
            # in matmul-consumption order so the first group unblocks early
            dgp_tiles, dgd_tiles = {}, {}
            for t in range(N_ETILES):
                for b in range(B):
                    for d in range(D):
                        dp = persist.tile([128, 128], bf16, name=f"dgp_{t}_{d}_{b}")
                        nc.vector.tensor_scalar_mul(dp, ident_t, xp_col(t, d, b))
                        dgp_tiles[(t, d, b)] = dp
                    for d in range(D):
                        dd = persist.tile([128, 128], bf16, name=f"dgd_{t}_{d}_{b}")
                        nc.vector.tensor_scalar_mul(dd, ident_t, xd_col(t, d, b))
                        dgd_tiles[(t, d, b)] = dd

            # ---- main loop ----
            for t in range(N_ETILES):
                for oc in range(N_OCHUNKS):
                    # 8 delay slices as 4 quarter-DMAs into one [p, (d o)]
                    # tile — quarter granularity lets matmuls start as soon
                    # as their d-slices land (Tile tracks subtile deps)
                    dm_all = stream.tile([128, D * OC], bf16, name="dm_all", bufs=8)
                    h = D // 4
                    base = t * 128 * N + oc * OC
                    for q in range(4):
                        nc.sync.dma_start(
                            out=dm_all[:, q * h * OC : (q + 1) * h * OC],
                            in_=bass.AP(
                                dmap_s,
                                base + q * h * E * N,
                                [[N, 128], [E * N, h], [1, OC]],
                            ),
                        )
                    # one DMA per tensor covering both batches: [p, (b o)]
                    boff = t * 128 * N + oc * OC
                    bdims = [[N, 128], [E * N, B], [1, OC]]
                    wtq = stream.tile([128, B * OC], u16, name="wtq", bufs=3)
                    nc.sync.dma_start(out=wtq, in_=bass.AP(W_s, boff, bdims).bitcast(u16))
                    apx2 = stream.tile([128, B * OC], bf16, name="apx2", bufs=3)
                    nc.scalar.dma_start(out=apx2, in_=bass.AP(ApX_s, boff, bdims))
                    adx2 = stream.tile([128, B * OC], bf16, name="adx2", bufs=3)
                    nc.scalar.dma_start(out=adx2, in_=bass.AP(AdX_s, boff, bdims))
                    u2 = outs.tile([128, B * OC], f32, name="u2", bufs=3)
                    uq2 = outs.tile([128, B * OC], u16, name="uq2", bufs=3)

                    ps = {}
                    for b in range(B):
                        ps[("p", b)] = psum.tile([128, OC], f32, name=f"ps_pot{b}")
                        ps[("d", b)] = psum.tile([128, OC], f32, name=f"ps_dep{b}")
                    # d-major interleave on the first block only (PE would
                    # otherwise starve while iter-0 quarters stream in);
                    # group-major elsewhere to keep PSUM lifetimes short
                    if t == 0 and oc <= 1:
                        for d in range(D):
                            rhs = dm_all[:, d * OC : (d + 1) * OC]
                            for b in range(B):
                                nc.tensor.matmul(
                                    ps[("p", b)], dgp_tiles[(t, d, b)], rhs,
                                    start=(d == 0), stop=(d == D - 1),
                                )
                                nc.tensor.matmul(
                                    ps[("d", b)], dgd_tiles[(t, d, b)], rhs,
                                    start=(d == 0), stop=(d == D - 1),
                                )
                    else:
                        for b in range(B):
                            for key, tiles in (("p", dgp_tiles), ("d", dgd_tiles)):
                                for d in range(D):
                                    nc.tensor.matmul(
                                        ps[(key, b)], tiles[(t, d, b)],
                                        dm_all[:, d * OC : (d + 1) * OC],
                                        start=(d == 0), stop=(d == D - 1),
                                    )

                    for b in range(B):
                        ps_pot = ps[("p", b)]
                        ps_dep = ps[("d", b)]
                        b_sl = slice(b * OC, (b + 1) * OC)
                        t1 = temps.tile([128, OC], f32, name="t1")
                        nc.vector.tensor_mul(t1, ps_pot, apx2[:, b_sl])
                        t2 = temps.tile([128, OC], f32, name="t2")
                        nc.vector.tensor_mul(t2, ps_dep, adx2[:, b_sl])
                        u = u2[:, b_sl]
                        nc.vector.tensor_add(u, wtq[:, b_sl], t1)
                        nc.vector.tensor_sub(u, u, t2)
                        uq = uq2[:, b_sl]
                        nc.vector.tensor_scalar(
                            out=uq,
                            in0=u,
                            scalar1=0.0,
                            scalar2=65535.0,
                            op0=mybir.AluOpType.max,
                            op1=mybir.AluOpType.min,
                        )
                        nc.scalar.dma_start(
                            out=bass.AP(
                                Wout_s,
                                b * E * N + t * 128 * N + oc * OC,
                                [[N, 128], [1, OC]],
                            ).bitcast(u16),
                            in_=uq,
                        )

    _split_waits(nc)
    return nc


_cache = {}


def _get_nc():
    if "nc" not in _cache:
        nc = bass.Bass("TRN2", target_bir_lowering=False, debug=False)
        _emit(nc)
        _cache["nc"] = nc
    return _cache["nc"]


def kernel(Xd, Xpost, xbar_pre, xbar_post, W, dmap, A_p, A_d):
    Xd = np.asarray(Xd, dtype=np.float32)
    Xpost = np.asarray(Xpost, dtype=np.float32)
    xbar_pre = np.asarray(xbar_pre, dtype=np.float32)
    xbar_post = np.asarray(xbar_post, dtype=np.float32)
    W = np.asarray(W, dtype=np.float32)
    dmap = np.asarray(dmap, dtype=np.float32)
    A_p = np.asarray(A_p, dtype=np.float32)
    A_d = np.asarray(A_d, dtype=np.float32)

    # fold the per-column masks into the amplitudes (host-side, cheap)
    ApX = (A_p[None, :, :] * Xpost[:, None, :] * 65535.0).astype(ml_dtypes.bfloat16)
    AdX = (A_d[None, :, :] * xbar_post[:, None, :] * 65535.0).astype(ml_dtypes.bfloat16)
    dmap_bf = dmap.astype(ml_dtypes.bfloat16)
    Wq = (np.clip(W, 0.0, 1.0) * 65535.0 + 0.5).astype(np.uint16)

    in_maps = []
    for k in range(M):
        sl = slice(k * E, (k + 1) * E)
        # packed per-core setup constant: [ident | xps | xds] f32
        stp = np.zeros((128, 128 + 2 * NDB), dtype=np.float32)
        stp[:, 0:128] = np.eye(128, dtype=np.float32)
        xp = xbar_pre[:, :, sl].reshape(D, B, N_ETILES, 128)
        stp[:, 128 : 128 + NDB] = xp.transpose(3, 2, 0, 1).reshape(128, NDB)
        xd = Xd[:, :, sl].reshape(D, B, N_ETILES, 128)
        stp[:, 128 + NDB :] = xd.transpose(3, 2, 0, 1).reshape(128, NDB)
        in_maps.append(
            {
                "dmap_s": np.ascontiguousarray(dmap_bf[:, sl, :]),
                "ApX_s": np.ascontiguousarray(ApX[:, sl, :]),
                "AdX_s": np.ascontiguousarray(AdX[:, sl, :]),
                "W_s": np.ascontiguousarray(Wq[:, sl, :]).view(np.float16),
                "setup": stp,
            }
        )

    nc = _get_nc()
    res = run_bass_kernel_spmd(nc, in_maps, core_ids=list(range(M)))

    W_new = np.concatenate(
        [res.results[k]["Wout_s"].view(np.uint16) for k in range(M)], axis=1
    ).astype(np.float32) * np.float32(1.0 / 65535.0)
    # tiny trace updates on host (exact, <0.1% of the data volume)
    xbar_pre_new = ALPHA_P * xbar_pre + (1.0 - ALPHA_P) * Xd
    xbar_post_new = ALPHA_D * xbar_post + (1.0 - ALPHA_D) * Xpost
    W_prev = W
    return W_prev, W_new, xbar_pre_new, xbar_post_new


# revision 112
# speedup vs baseline: 1.0017x; 1.0017x over previous
"""Abbott STDP step kernel for 8 Trainium2 NeuronCores.

Math (per reference):
  dW_pot[b,e,o] = Xpost[b,o]   * (sum_d xbar_pre[d,b,e]*dmap[d,e,o]) * A_p[e,o]
  dW_dep[b,e,o] = xbar_post[b,o]* (sum_d Xd[d,b,e]      *dmap[d,e,o]) * A_d[e,o]
  W_new = clip(W + dW_pot - dW_dep, 0, 1)
  xbar_pre_new  = 0.95*xbar_pre  + 0.05*Xd      (host; trivially small)
  xbar_post_new = 0.90*xbar_post + 0.10*Xpost   (host; trivially small)

Sharding: presynaptic axis e (axis 1 of dmap/W/A, axis 2 of xbar_pre/Xd)
split into 8 slabs of 256; no cross-device reduction (d is the only
contracted axis).

Per-core compute:
  - d-contraction on the TensorEngine: 8 PSUM-accumulated matmuls per
    (b, pot/dep) output tile with diag(xbar[d,b,.]) stationary, in bf16
    (1 cycle/row; fp32 PSUM accumulation).  The diagonals are built once
    on DVE from a single packed host constant [ident | xps | xds].
  - The per-column masks and amplitudes are folded on the host into
    ApX[b]=A_p*Xpost[b]*65535, AdX[b]=A_d*xbar_post[b]*65535 (bf16), so
    the on-chip combine is just: t1=S_pot*ApX, t2=S_dep*AdX (DVE, PSUM
    source), u=W+t1-t2 (the add reads the u16 W tile directly; DVE
    converts inline, exactly), clip to [0,65535] (DVE fused max/min
    tensor_scalar casting to uint16).
  - W rides in 16-bit fixed point (W*65535 as uint16, exact to 7.6e-6
    since W_new is clipped to [0,1]): halves the W round-trip traffic.
    The u16 buffers wear a float16 facade at the DRAM/PJRT boundary
    (PJRT cannot bind u16); APs are bitcast inside the kernel.
  - dmap streams as quartered [128, 8*512] bf16 DMAs per (e-tile,
    o-chunk) on the SP HWDGE ring with 8-deep prefetch; W rides the
    same ring; ApX/AdX loads and W_new writes ride the ACT ring.
"""

import ml_dtypes
import numpy as np

import concourse.bass as bass
import concourse.mybir as mybir
from concourse.tile import TileContext
from concourse.bass_utils import run_bass_kernel_spmd

D, B, N = 8, 2, 2048
M = 8                  # cores
E = N // M             # 256 — per-core e-slab
OC = 512               # o chunk width
N_ETILES = E // 128    # 2
N_OCHUNKS = N // OC    # 4
ALPHA_P, ALPHA_D = 0.95, 0.9
WMAX = 1.0
NDB = N_ETILES * D * B  # 32 scalar columns per family

f32 = mybir.dt.float32
bf16 = mybir.dt.bfloat16
u16 = mybir.dt.uint16
f16 = mybir.dt.float16


def _split_waits(nc: bass.Bass):
    """Hoist all but one sem wait per instruction onto standalone
    same-engine InstNoOp carriers placed immediately before it.

    The walrus build used here rejects any instruction carrying more
    than one sync wait ("Too many sync wait commands"); engines execute
    in order, so a preceding same-engine wait is equivalent.
    """
    ctr = 0
    for f in nc.m.functions:
        for blk in f.blocks:
            il = blk.instructions
            i = 0
            while i < len(il):
                inst = il[i]
                si = inst.sync_info
                if si is not None and si.on_wait and len(si.on_wait) > 1:
                    waits = list(si.on_wait)
                    inst.sync_info = mybir.SyncInfo(
                        on_wait=[waits[0]], on_update=list(si.on_update or [])
                    )
                    for w in waits[1:]:
                        nop = mybir.InstNoOp(name=f"wsplit-{ctr}", ins=[], outs=[])
                        ctr += 1
                        nop.engine = inst.engine
                        nop.sync_info = mybir.SyncInfo(on_wait=[w], on_update=[])
                        il.insert(i, nop)
                        i += 1
                i += 1


def _emit(nc: bass.Bass):
    dmap_s = nc.dram_tensor("dmap_s", [D, E, N], bf16, kind="ExternalInput")
    ApX_s = nc.dram_tensor("ApX_s", [B, E, N], bf16, kind="ExternalInput")
    AdX_s = nc.dram_tensor("AdX_s", [B, E, N], bf16, kind="ExternalInput")
    W_s = nc.dram_tensor("W_s", [B, E, N], f16, kind="ExternalInput")
    setup = nc.dram_tensor("setup", [128, 128 + 2 * NDB], f32, kind="ExternalInput")

    Wout_s = nc.dram_tensor("Wout_s", [B, E, N], f16, kind="ExternalOutput")

    with TileContext(nc) as tc:
        with (
            tc.tile_pool(name="persist", bufs=1) as persist,
            tc.tile_pool(name="stream", bufs=2) as stream,
            tc.tile_pool(name="temps", bufs=4) as temps,
            tc.tile_pool(name="outs", bufs=4) as outs,
            tc.tile_pool(name="psum", bufs=2, space="PSUM") as psum,
        ):
            # ---- one-time setup: one small DMA at the head of the SP ring ----
            setup_t = persist.tile([128, 128 + 2 * NDB], f32, name="setup_t")
            nc.sync.dma_start(out=setup_t, in_=setup[:, :])
            ident_t = setup_t[:, 0:128]

            def xp_col(t, d, b):
                c = 128 + t * 16 + d * 2 + b
                return setup_t[:, c : c + 1]

            def xd_col(t, d, b):
                c = 128 + NDB + t * 16 + d * 2 + b
                return setup_t[:, c : c + 1]

            # diag matrices diag(xbar_pre[d,b,e_tile]), diag(Xd[d,b,e_tile]),
            # in matmul-consumption order so the first group unblocks early
            dgp_tiles, dgd_tiles = {}, {}
            for t in range(N_ETILES):
                for b in range(B):
                    for d in range(D):
                        dp = persist.tile([128, 128], bf16, name=f"dgp_{t}_{d}_{b}")
                        nc.vector.tensor_scalar_mul(dp, ident_t, xp_col(t, d, b))
                        dgp_tiles[(t, d, b)] = dp
                    for d in range(D):
                        dd = persist.tile([128, 128], bf16, name=f"dgd_{t}_{d}_{b}")
                        nc.vector.tensor_scalar_mul(dd, ident_t, xd_col(t, d, b))
                        dgd_tiles[(t, d, b)] = dd

            # ---- main loop ----
            for t in range(N_ETILES):
                for oc in range(N_OCHUNKS):
                    # 8 delay slices as 4 quarter-DMAs into one [p, (d o)]
                    # tile — quarter granularity lets matmuls start as soon
                    # as their d-slices land (Tile tracks subtile deps)
                    dm_all = stream.tile([128, D * OC], bf16, name="dm_all", bufs=8)
                    h = D // 4
                    base = t * 128 * N + oc * OC
                    for q in range(4):
                        nc.sync.dma_start(
                            out=dm_all[:, q * h * OC : (q + 1) * h * OC],
                            in_=bass.AP(
                                dmap_s,
                                base + q * h * E * N,
                                [[N, 128], [E * N, h], [1, OC]],
                            ),
                        )
                    # one DMA per tensor covering both batches: [p, (b o)]
                    boff = t * 128 * N + oc * OC
                    bdims = [[N, 128], [E * N, B], [1, OC]]
                    wtq = stream.tile([128, B * OC], u16, name="wtq", bufs=3)
                    nc.sync.dma_start(out=wtq, in_=bass.AP(W_s, boff, bdims).bitcast(u16))
                    apx2 = stream.tile([128, B * OC], bf16, name="apx2", bufs=3)
                    nc.scalar.dma_start(out=apx2, in_=bass.AP(ApX_s, boff, bdims))
                    adx2 = stream.tile([128, B * OC], bf16, name="adx2", bufs=3)
                    nc.scalar.dma_start(out=adx2, in_=bass.AP(AdX_s, boff, bdims))
                    u2 = outs.tile([128, B * OC], f32, name="u2", bufs=3)
                    uq2 = outs.tile([128, B * OC], u16, name="uq2", bufs=3)

                    ps = {}
                    for b in range(B):
                        ps[("p", b)] = psum.tile([128, OC], f32, name=f"ps_pot{b}")
                        ps[("d", b)] = psum.tile([128, OC], f32, name=f"ps_dep{b}")
                    # d-major interleave on the first block only (PE would
                    # otherwise starve while iter-0 quarters stream in);
                    # group-major elsewhere to keep PSUM lifetimes short
                    if t == 0 and oc <= 1:
                        for d in range(D):
                            rhs = dm_all[:, d * OC : (d + 1) * OC]
                            for b in range(B):
                                nc.tensor.matmul(
                                    ps[("p", b)], dgp_tiles[(t, d, b)], rhs,
                                    start=(d == 0), stop=(d == D - 1),
                                )
                                nc.tensor.matmul(
                                    ps[("d", b)], dgd_tiles[(t, d, b)], rhs,
                                    start=(d == 0), stop=(d == D - 1),
                                )
                    else:
                        for b in range(B):
                            for key, tiles in (("p", dgp_tiles), ("d", dgd_tiles)):
                                for d in range(D):
                                    nc.tensor.matmul(
                                        ps[(key, b)], tiles[(t, d, b)],
                                        dm_all[:, d * OC : (d + 1) * OC],
                                        start=(d == 0), stop=(d == D - 1),
                                    )

                    for b in range(B):
                        ps_pot = ps[("p", b)]
                        ps_dep = ps[("d", b)]
                        b_sl = slice(b * OC, (b + 1) * OC)
                        t1 = temps.tile([128, OC], f32, name="t1")
                        nc.vector.tensor_mul(t1, ps_pot, apx2[:, b_sl])
                        t2 = temps.tile([128, OC], f32, name="t2")
                        nc.vector.tensor_mul(t2, ps_dep, adx2[:, b_sl])
                        u = u2[:, b_sl]
                        nc.vector.tensor_add(u, wtq[:, b_sl], t1)
                        nc.vector.tensor_sub(u, u, t2)
                        uq = uq2[:, b_sl]
                        nc.vector.tensor_scalar(
                            out=uq,
                            in0=u,
                            scalar1=0.0,
                            scalar2=65535.0,
                            op0=mybir.AluOpType.max,
                            op1=mybir.AluOpType.min,
                        )
                        nc.scalar.dma_start(
                            out=bass.AP(
                                Wout_s,
                                b * E * N + t * 128 * N + oc * OC,
                                [[N, 128], [1, OC]],
                            ).bitcast(u16),
                            in_=uq,
                        )

    _split_waits(nc)
    return nc


_cache = {}


def _get_nc():
    if "nc" not in _cache:
        nc = bass.Bass("TRN2", target_bir_lowering=False, debug=False)
        _emit(nc)
        _cache["nc"] = nc
    return _cache["nc"]


def kernel(Xd, Xpost, xbar_pre, xbar_post, W, dmap, A_p, A_d):
    Xd = np.asarray(Xd, dtype=np.float32)
    Xpost = np.asarray(Xpost, dtype=np.float32)
    xbar_pre = np.asarray(xbar_pre, dtype=np.float32)
    xbar_post = np.asarray(xbar_post, dtype=np.float32)
    W = np.asarray(W, dtype=np.float32)
    dmap = np.asarray(dmap, dtype=np.float32)
    A_p = np.asarray(A_p, dtype=np.float32)
    A_d = np.asarray(A_d, dtype=np.float32)

    # fold the per-column masks into the amplitudes (host-side, cheap)
    ApX = (A_p[None, :, :] * Xpost[:, None, :] * 65535.0).astype(ml_dtypes.bfloat16)
    AdX = (A_d[None, :, :] * xbar_post[:, None, :] * 65535.0).astype(ml_dtypes.bfloat16)
    dmap_bf = dmap.astype(ml_dtypes.bfloat16)
    Wq = (np.clip(W, 0.0, 1.0) * 65535.0 + 0.5).astype(np.uint16)

    in_maps = []
    for k in range(M):
        sl = slice(k * E, (k + 1) * E)
        # packed per-core setup constant: [ident | xps | xds] f32
        stp = np.zeros((128, 128 + 2 * NDB), dtype=np.float32)
        stp[:, 0:128] = np.eye(128, dtype=np.float32)
        xp = xbar_pre[:, :, sl].reshape(D, B, N_ETILES, 128)
        stp[:, 128 : 128 + NDB] = xp.transpose(3, 2, 0, 1).reshape(128, NDB)
        xd = Xd[:, :, sl].reshape(D, B, N_ETILES, 128)
        stp[:, 128 + NDB :] = xd.transpose(3, 2, 0, 1).reshape(128, NDB)
        in_maps.append(
            {
                "dmap_s": np.ascontiguousarray(dmap_bf[:, sl, :]),
                "ApX_s": np.ascontiguousarray(ApX[:, sl, :]),
                "AdX_s": np.ascontiguousarray(AdX[:, sl, :]),
                "W_s": np.ascontiguousarray(Wq[:, sl, :]).view(np.float16),
                "setup": stp,
            }
        )

    nc = _get_nc()
    res = run_bass_kernel_spmd(nc, in_maps, core_ids=list(range(M)))

    W_new = np.concatenate(
        [res.results[k]["Wout_s"].view(np.uint16) for k in range(M)], axis=1
    ).astype(np.float32) * np.float32(1.0 / 65535.0)
    # tiny trace updates on host (exact, <0.1% of the data volume)
    xbar_pre_new = ALPHA_P * xbar_pre + (1.0 - ALPHA_P) * Xd
    xbar_post_new = ALPHA_D * xbar_post + (1.0 - ALPHA_D) * Xpost
    W_prev = W
    return W_prev, W_new, xbar_pre_new, xbar_post_new


# revision 113
# speedup vs baseline: 1.0063x; 1.0046x over previous
"""Abbott STDP step kernel for 8 Trainium2 NeuronCores.

Math (per reference):
  dW_pot[b,e,o] = Xpost[b,o]   * (sum_d xbar_pre[d,b,e]*dmap[d,e,o]) * A_p[e,o]
  dW_dep[b,e,o] = xbar_post[b,o]* (sum_d Xd[d,b,e]      *dmap[d,e,o]) * A_d[e,o]
  W_new = clip(W + dW_pot - dW_dep, 0, 1)
  xbar_pre_new  = 0.95*xbar_pre  + 0.05*Xd      (host; trivially small)
  xbar_post_new = 0.90*xbar_post + 0.10*Xpost   (host; trivially small)

Sharding: presynaptic axis e (axis 1 of dmap/W/A, axis 2 of xbar_pre/Xd)
split into 8 slabs of 256; no cross-device reduction (d is the only
contracted axis).

Per-core compute:
  - d-contraction on the TensorEngine: 8 PSUM-accumulated matmuls per
    (b, pot/dep) output tile with diag(xbar[d,b,.]) stationary, in bf16
    (1 cycle/row; fp32 PSUM accumulation).  The diagonals are built once
    on DVE from a single packed host constant [ident | xps | xds].
  - The per-column masks and amplitudes are folded on the host into
    ApX[b]=A_p*Xpost[b]*65535, AdX[b]=A_d*xbar_post[b]*65535 (bf16), so
    the on-chip combine is just: t1=S_pot*ApX, t2=S_dep*AdX (DVE, PSUM
    source), u=W+t1-t2 (the add reads the u16 W tile directly; DVE
    converts inline, exactly), clip to [0,65535] (DVE fused max/min
    tensor_scalar casting to uint16).
  - W rides in 16-bit fixed point (W*65535 as uint16, exact to 7.6e-6
    since W_new is clipped to [0,1]): halves the W round-trip traffic.
    The u16 buffers wear a float16 facade at the DRAM/PJRT boundary
    (PJRT cannot bind u16); APs are bitcast inside the kernel.
  - dmap streams as quartered [128, 8*512] bf16 DMAs per (e-tile,
    o-chunk) on the SP HWDGE ring with 8-deep prefetch; W rides the
    same ring; ApX/AdX loads and W_new writes ride the ACT ring.
"""

import ml_dtypes
import numpy as np

import concourse.bass as bass
import concourse.mybir as mybir
from concourse.tile import TileContext
from concourse.bass_utils import run_bass_kernel_spmd

D, B, N = 8, 2, 2048
M = 8                  # cores
E = N // M             # 256 — per-core e-slab
OC = 512               # o chunk width
N_ETILES = E // 128    # 2
N_OCHUNKS = N // OC    # 4
ALPHA_P, ALPHA_D = 0.95, 0.9
WMAX = 1.0
NDB = N_ETILES * D * B  # 32 scalar columns per family

f32 = mybir.dt.float32
bf16 = mybir.dt.bfloat16
u16 = mybir.dt.uint16
f16 = mybir.dt.float16


def _split_waits(nc: bass.Bass):
    """Hoist all but one sem wait per instruction onto standalone
    same-engine InstNoOp carriers placed immediately before it.

    The walrus build used here rejects any instruction carrying more
    than one sync wait ("Too many sync wait commands"); engines execute
    in order, so a preceding same-engine wait is equivalent.
    """
    ctr = 0
    for f in nc.m.functions:
        for blk in f.blocks:
            il = blk.instructions
            i = 0
            while i < len(il):
                inst = il[i]
                si = inst.sync_info
                if si is not None and si.on_wait and len(si.on_wait) > 1:
                    waits = list(si.on_wait)
                    inst.sync_info = mybir.SyncInfo(
                        on_wait=[waits[0]], on_update=list(si.on_update or [])
                    )
                    for w in reversed(waits[1:]):
                        nop = mybir.InstNoOp(name=f"wsplit-{ctr}", ins=[], outs=[])
                        ctr += 1
                        nop.engine = inst.engine
                        nop.sync_info = mybir.SyncInfo(on_wait=[w], on_update=[])
                        il.insert(i, nop)
                        i += 1
                i += 1


def _emit(nc: bass.Bass):
    dmap_s = nc.dram_tensor("dmap_s", [D, E, N], bf16, kind="ExternalInput")
    ApX_s = nc.dram_tensor("ApX_s", [B, E, N], bf16, kind="ExternalInput")
    AdX_s = nc.dram_tensor("AdX_s", [B, E, N], bf16, kind="ExternalInput")
    W_s = nc.dram_tensor("W_s", [B, E, N], f16, kind="ExternalInput")
    setup = nc.dram_tensor("setup", [128, 128 + 2 * NDB], f32, kind="ExternalInput")

    Wout_s = nc.dram_tensor("Wout_s", [B, E, N], f16, kind="ExternalOutput")

    with TileContext(nc) as tc:
        with (
            tc.tile_pool(name="persist", bufs=1) as persist,
            tc.tile_pool(name="stream", bufs=2) as stream,
            tc.tile_pool(name="temps", bufs=4) as temps,
            tc.tile_pool(name="outs", bufs=4) as outs,
            tc.tile_pool(name="psum", bufs=2, space="PSUM") as psum,
        ):
            # ---- one-time setup: one small DMA at the head of the SP ring ----
            setup_t = persist.tile([128, 128 + 2 * NDB], f32, name="setup_t")
            nc.sync.dma_start(out=setup_t, in_=setup[:, :])
            ident_t = setup_t[:, 0:128]

            def xp_col(t, d, b):
                c = 128 + t * 16 + d * 2 + b
                return setup_t[:, c : c + 1]

            def xd_col(t, d, b):
                c = 128 + NDB + t * 16 + d * 2 + b
                return setup_t[:, c : c + 1]

            # diag matrices diag(xbar_pre[d,b,e_tile]), diag(Xd[d,b,e_tile]),
            # in matmul-consumption order so the first group unblocks early
            dgp_tiles, dgd_tiles = {}, {}
            for t in range(N_ETILES):
                for b in range(B):
                    for d in range(D):
                        dp = persist.tile([128, 128], bf16, name=f"dgp_{t}_{d}_{b}")
                        nc.vector.tensor_scalar_mul(dp, ident_t, xp_col(t, d, b))
                        dgp_tiles[(t, d, b)] = dp
                    for d in range(D):
                        dd = persist.tile([128, 128], bf16, name=f"dgd_{t}_{d}_{b}")
                        nc.vector.tensor_scalar_mul(dd, ident_t, xd_col(t, d, b))
                        dgd_tiles[(t, d, b)] = dd

            # ---- main loop ----
            for t in range(N_ETILES):
                for oc in range(N_OCHUNKS):
                    # 8 delay slices as 4 quarter-DMAs into one [p, (d o)]
                    # tile — quarter granularity lets matmuls start as soon
                    # as their d-slices land (Tile tracks subtile deps)
                    dm_all = stream.tile([128, D * OC], bf16, name="dm_all", bufs=8)
                    h = D // 4
                    base = t * 128 * N + oc * OC
                    for q in range(4):
                        nc.sync.dma_start(
                            out=dm_all[:, q * h * OC : (q + 1) * h * OC],
                            in_=bass.AP(
                                dmap_s,
                                base + q * h * E * N,
                                [[N, 128], [E * N, h], [1, OC]],
                            ),
                        )
                    # one DMA per tensor covering both batches: [p, (b o)]
                    boff = t * 128 * N + oc * OC
                    bdims = [[N, 128], [E * N, B], [1, OC]]
                    wtq = stream.tile([128, B * OC], u16, name="wtq", bufs=3)
                    nc.sync.dma_start(out=wtq, in_=bass.AP(W_s, boff, bdims).bitcast(u16))
                    apx2 = stream.tile([128, B * OC], bf16, name="apx2", bufs=3)
                    nc.scalar.dma_start(out=apx2, in_=bass.AP(ApX_s, boff, bdims))
                    adx2 = stream.tile([128, B * OC], bf16, name="adx2", bufs=3)
                    nc.scalar.dma_start(out=adx2, in_=bass.AP(AdX_s, boff, bdims))
                    u2 = outs.tile([128, B * OC], f32, name="u2", bufs=3)
                    uq2 = outs.tile([128, B * OC], u16, name="uq2", bufs=3)

                    ps = {}
                    for b in range(B):
                        ps[("p", b)] = psum.tile([128, OC], f32, name=f"ps_pot{b}")
                        ps[("d", b)] = psum.tile([128, OC], f32, name=f"ps_dep{b}")
                    # d-major interleave on the first block only (PE would
                    # otherwise starve while iter-0 quarters stream in);
                    # group-major elsewhere to keep PSUM lifetimes short
                    if t == 0 and oc <= 1:
                        for d in range(D):
                            rhs = dm_all[:, d * OC : (d + 1) * OC]
                            for b in range(B):
                                nc.tensor.matmul(
                                    ps[("p", b)], dgp_tiles[(t, d, b)], rhs,
                                    start=(d == 0), stop=(d == D - 1),
                                )
                                nc.tensor.matmul(
                                    ps[("d", b)], dgd_tiles[(t, d, b)], rhs,
                                    start=(d == 0), stop=(d == D - 1),
                                )
                    else:
                        for b in range(B):
                            for key, tiles in (("p", dgp_tiles), ("d", dgd_tiles)):
                                for d in range(D):
                                    nc.tensor.matmul(
                                        ps[(key, b)], tiles[(t, d, b)],
                                        dm_all[:, d * OC : (d + 1) * OC],
                                        start=(d == 0), stop=(d == D - 1),
                                    )

                    for b in range(B):
                        ps_pot = ps[("p", b)]
                        ps_dep = ps[("d", b)]
                        b_sl = slice(b * OC, (b + 1) * OC)
                        t1 = temps.tile([128, OC], f32, name="t1")
                        nc.vector.tensor_mul(t1, ps_pot, apx2[:, b_sl])
                        t2 = temps.tile([128, OC], f32, name="t2")
                        nc.vector.tensor_mul(t2, ps_dep, adx2[:, b_sl])
                        u = u2[:, b_sl]
                        nc.vector.tensor_add(u, wtq[:, b_sl], t1)
                        nc.vector.tensor_sub(u, u, t2)
                        uq = uq2[:, b_sl]
                        nc.vector.tensor_scalar(
                            out=uq,
                            in0=u,
                            scalar1=0.0,
                            scalar2=65535.0,
                            op0=mybir.AluOpType.max,
                            op1=mybir.AluOpType.min,
                        )
                        nc.scalar.dma_start(
                            out=bass.AP(
                                Wout_s,
                                b * E * N + t * 128 * N + oc * OC,
                                [[N, 128], [1, OC]],
                            ).bitcast(u16),
                            in_=uq,
                        )

    _split_waits(nc)
    return nc


_cache = {}


def _get_nc():
    if "nc" not in _cache:
        nc = bass.Bass("TRN2", target_bir_lowering=False, debug=False)
        _emit(nc)
        _cache["nc"] = nc
    return _cache["nc"]


def kernel(Xd, Xpost, xbar_pre, xbar_post, W, dmap, A_p, A_d):
    Xd = np.asarray(Xd, dtype=np.float32)
    Xpost = np.asarray(Xpost, dtype=np.float32)
    xbar_pre = np.asarray(xbar_pre, dtype=np.float32)
    xbar_post = np.asarray(xbar_post, dtype=np.float32)
    W = np.asarray(W, dtype=np.float32)
    dmap = np.asarray(dmap, dtype=np.float32)
    A_p = np.asarray(A_p, dtype=np.float32)
    A_d = np.asarray(A_d, dtype=np.float32)

    # fold the per-column masks into the amplitudes (host-side, cheap)
    ApX = (A_p[None, :, :] * Xpost[:, None, :] * 65535.0).astype(ml_dtypes.bfloat16)
    AdX = (A_d[None, :, :] * xbar_post[:, None, :] * 65535.0).astype(ml_dtypes.bfloat16)
    dmap_bf = dmap.astype(ml_dtypes.bfloat16)
    Wq = (np.clip(W, 0.0, 1.0) * 65535.0 + 0.5).astype(np.uint16)

    in_maps = []
    for k in range(M):
        sl = slice(k * E, (k + 1) * E)
        # packed per-core setup constant: [ident | xps | xds] f32
        stp = np.zeros((128, 128 + 2 * NDB), dtype=np.float32)
        stp[:, 0:128] = np.eye(128, dtype=np.float32)
        xp = xbar_pre[:, :, sl].reshape(D, B, N_ETILES, 128)
        stp[:, 128 : 128 + NDB] = xp.transpose(3, 2, 0, 1).reshape(128, NDB)
        xd = Xd[:, :, sl].reshape(D, B, N_ETILES, 128)
        stp[:, 128 + NDB :] = xd.transpose(3, 2, 0, 1).reshape(128, NDB)
        in_maps.append(
            {
                "dmap_s": np.ascontiguousarray(dmap_bf[:, sl, :]),
                "ApX_s": np.ascontiguousarray(ApX[:, sl, :]),
                "AdX_s": np.ascontiguousarray(AdX[:, sl, :]),
                "W_s": np.ascontiguousarray(Wq[:, sl, :]).view(np.float16),
                "setup": stp,
            }
        )

    nc = _get_nc()
    res = run_bass_kernel_spmd(nc, in_maps, core_ids=list(range(M)))

    W_new = np.concatenate(
        [res.results[k]["Wout_s"].view(np.uint16) for k in range(M)], axis=1
    ).astype(np.float32) * np.float32(1.0 / 65535.0)
    # tiny trace updates on host (exact, <0.1% of the data volume)
    xbar_pre_new = ALPHA_P * xbar_pre + (1.0 - ALPHA_P) * Xd
    xbar_post_new = ALPHA_D * xbar_post + (1.0 - ALPHA_D) * Xpost
    W_prev = W
    return W_prev, W_new, xbar_pre_new, xbar_post_new


# revision 115
# speedup vs baseline: 1.0070x; 1.0008x over previous
"""Abbott STDP step kernel for 8 Trainium2 NeuronCores.

Math (per reference):
  dW_pot[b,e,o] = Xpost[b,o]   * (sum_d xbar_pre[d,b,e]*dmap[d,e,o]) * A_p[e,o]
  dW_dep[b,e,o] = xbar_post[b,o]* (sum_d Xd[d,b,e]      *dmap[d,e,o]) * A_d[e,o]
  W_new = clip(W + dW_pot - dW_dep, 0, 1)
  xbar_pre_new  = 0.95*xbar_pre  + 0.05*Xd      (host; trivially small)
  xbar_post_new = 0.90*xbar_post + 0.10*Xpost   (host; trivially small)

Sharding: presynaptic axis e (axis 1 of dmap/W/A, axis 2 of xbar_pre/Xd)
split into 8 slabs of 256; no cross-device reduction (d is the only
contracted axis).

Per-core compute:
  - d-contraction on the TensorEngine: 8 PSUM-accumulated matmuls per
    (b, pot/dep) output tile with diag(xbar[d,b,.]) stationary, in bf16
    (1 cycle/row; fp32 PSUM accumulation).  The diagonals are built once
    on DVE from a single packed host constant [ident | xps | xds].
  - The per-column masks and amplitudes are folded on the host into
    ApX[b]=A_p*Xpost[b]*65535, AdX[b]=A_d*xbar_post[b]*65535 (bf16), so
    the on-chip combine is just: t1=S_pot*ApX, t2=S_dep*AdX (DVE, PSUM
    source), u=W+t1-t2 (the add reads the u16 W tile directly; DVE
    converts inline, exactly), clip to [0,65535] (DVE fused max/min
    tensor_scalar casting to uint16).
  - W rides in 16-bit fixed point (W*65535 as uint16, exact to 7.6e-6
    since W_new is clipped to [0,1]): halves the W round-trip traffic.
    The u16 buffers wear a float16 facade at the DRAM/PJRT boundary
    (PJRT cannot bind u16); APs are bitcast inside the kernel.
  - dmap streams as quartered [128, 8*512] bf16 DMAs per (e-tile,
    o-chunk) on the SP HWDGE ring with 8-deep prefetch; W rides the
    same ring; ApX/AdX loads and W_new writes ride the ACT ring.
"""

import ml_dtypes
import numpy as np

import concourse.bass as bass
import concourse.mybir as mybir
from concourse.tile import TileContext
from concourse.bass_utils import run_bass_kernel_spmd

D, B, N = 8, 2, 2048
M = 8                  # cores
E = N // M             # 256 — per-core e-slab
OC = 512               # o chunk width
N_ETILES = E // 128    # 2
N_OCHUNKS = N // OC    # 4
ALPHA_P, ALPHA_D = 0.95, 0.9
WMAX = 1.0
NDB = N_ETILES * D * B  # 32 scalar columns per family

f32 = mybir.dt.float32
bf16 = mybir.dt.bfloat16
u16 = mybir.dt.uint16
f16 = mybir.dt.float16


def _split_waits(nc: bass.Bass):
    """Hoist all but one sem wait per instruction onto standalone
    same-engine InstNoOp carriers placed immediately before it.

    The walrus build used here rejects any instruction carrying more
    than one sync wait ("Too many sync wait commands"); engines execute
    in order, so a preceding same-engine wait is equivalent.
    """
    ctr = 0
    for f in nc.m.functions:
        for blk in f.blocks:
            il = blk.instructions
            i = 0
            while i < len(il):
                inst = il[i]
                si = inst.sync_info
                if si is not None and si.on_wait and len(si.on_wait) > 1:
                    waits = list(si.on_wait)
                    inst.sync_info = mybir.SyncInfo(
                        on_wait=[waits[0]], on_update=list(si.on_update or [])
                    )
                    for w in sorted(waits[1:], key=lambda x: -x.id):
                        nop = mybir.InstNoOp(name=f"wsplit-{ctr}", ins=[], outs=[])
                        ctr += 1
                        nop.engine = inst.engine
                        nop.sync_info = mybir.SyncInfo(on_wait=[w], on_update=[])
                        il.insert(i, nop)
                        i += 1
                i += 1


def _emit(nc: bass.Bass):
    dmap_s = nc.dram_tensor("dmap_s", [D, E, N], bf16, kind="ExternalInput")
    ApX_s = nc.dram_tensor("ApX_s", [B, E, N], bf16, kind="ExternalInput")
    AdX_s = nc.dram_tensor("AdX_s", [B, E, N], bf16, kind="ExternalInput")
    W_s = nc.dram_tensor("W_s", [B, E, N], f16, kind="ExternalInput")
    setup = nc.dram_tensor("setup", [128, 128 + 2 * NDB], f32, kind="ExternalInput")

    Wout_s = nc.dram_tensor("Wout_s", [B, E, N], f16, kind="ExternalOutput")

    with TileContext(nc) as tc:
        with (
            tc.tile_pool(name="persist", bufs=1) as persist,
            tc.tile_pool(name="stream", bufs=2) as stream,
            tc.tile_pool(name="temps", bufs=4) as temps,
            tc.tile_pool(name="outs", bufs=4) as outs,
            tc.tile_pool(name="psum", bufs=2, space="PSUM") as psum,
        ):
            # ---- one-time setup: one small DMA at the head of the SP ring ----
            setup_t = persist.tile([128, 128 + 2 * NDB], f32, name="setup_t")
            nc.sync.dma_start(out=setup_t, in_=setup[:, :])
            ident_t = setup_t[:, 0:128]

            def xp_col(t, d, b):
                c = 128 + t * 16 + d * 2 + b
                return setup_t[:, c : c + 1]

            def xd_col(t, d, b):
                c = 128 + NDB + t * 16 + d * 2 + b
                return setup_t[:, c : c + 1]

            # diag matrices diag(xbar_pre[d,b,e_tile]), diag(Xd[d,b,e_tile]),
            # in matmul-consumption order so the first group unblocks early
            dgp_tiles, dgd_tiles = {}, {}
            for t in range(N_ETILES):
                for b in range(B):
                    for d in range(D):
                        dp = persist.tile([128, 128], bf16, name=f"dgp_{t}_{d}_{b}")
                        nc.vector.tensor_scalar_mul(dp, ident_t, xp_col(t, d, b))
                        dgp_tiles[(t, d, b)] = dp
                    for d in range(D):
                        dd = persist.tile([128, 128], bf16, name=f"dgd_{t}_{d}_{b}")
                        nc.vector.tensor_scalar_mul(dd, ident_t, xd_col(t, d, b))
                        dgd_tiles[(t, d, b)] = dd

            # ---- main loop ----
            for t in range(N_ETILES):
                for oc in range(N_OCHUNKS):
                    # 8 delay slices as 4 quarter-DMAs into one [p, (d o)]
                    # tile — quarter granularity lets matmuls start as soon
                    # as their d-slices land (Tile tracks subtile deps)
                    dm_all = stream.tile([128, D * OC], bf16, name="dm_all", bufs=8)
                    h = D // 4
                    base = t * 128 * N + oc * OC
                    for q in range(4):
                        nc.sync.dma_start(
                            out=dm_all[:, q * h * OC : (q + 1) * h * OC],
                            in_=bass.AP(
                                dmap_s,
                                base + q * h * E * N,
                                [[N, 128], [E * N, h], [1, OC]],
                            ),
                        )
                    # one DMA per tensor covering both batches: [p, (b o)]
                    boff = t * 128 * N + oc * OC
                    bdims = [[N, 128], [E * N, B], [1, OC]]
                    wtq = stream.tile([128, B * OC], u16, name="wtq", bufs=3)
                    nc.sync.dma_start(out=wtq, in_=bass.AP(W_s, boff, bdims).bitcast(u16))
                    apx2 = stream.tile([128, B * OC], bf16, name="apx2", bufs=3)
                    nc.scalar.dma_start(out=apx2, in_=bass.AP(ApX_s, boff, bdims))
                    adx2 = stream.tile([128, B * OC], bf16, name="adx2", bufs=3)
                    nc.scalar.dma_start(out=adx2, in_=bass.AP(AdX_s, boff, bdims))
                    u2 = outs.tile([128, B * OC], f32, name="u2", bufs=3)
                    uq2 = outs.tile([128, B * OC], u16, name="uq2", bufs=3)

                    ps = {}
                    for b in range(B):
                        ps[("p", b)] = psum.tile([128, OC], f32, name=f"ps_pot{b}")
                        ps[("d", b)] = psum.tile([128, OC], f32, name=f"ps_dep{b}")
                    # d-major interleave on the first block only (PE would
                    # otherwise starve while iter-0 quarters stream in);
                    # group-major elsewhere to keep PSUM lifetimes short
                    if t == 0 and oc <= 1:
                        for d in range(D):
                            rhs = dm_all[:, d * OC : (d + 1) * OC]
                            for b in range(B):
                                nc.tensor.matmul(
                                    ps[("p", b)], dgp_tiles[(t, d, b)], rhs,
                                    start=(d == 0), stop=(d == D - 1),
                                )
                                nc.tensor.matmul(
                                    ps[("d", b)], dgd_tiles[(t, d, b)], rhs,
                                    start=(d == 0), stop=(d == D - 1),
                                )
                    else:
                        for b in range(B):
                            for key, tiles in (("p", dgp_tiles), ("d", dgd_tiles)):
                                for d in range(D):
                                    nc.tensor.matmul(
                                        ps[(key, b)], tiles[(t, d, b)],
                                        dm_all[:, d * OC : (d + 1) * OC],
                                        start=(d == 0), stop=(d == D - 1),
                                    )

                    for b in range(B):
                        ps_pot = ps[("p", b)]
                        ps_dep = ps[("d", b)]
                        b_sl = slice(b * OC, (b + 1) * OC)
                        t1 = temps.tile([128, OC], f32, name="t1")
                        nc.vector.tensor_mul(t1, ps_pot, apx2[:, b_sl])
                        t2 = temps.tile([128, OC], f32, name="t2")
                        nc.vector.tensor_mul(t2, ps_dep, adx2[:, b_sl])
                        u = u2[:, b_sl]
                        nc.vector.tensor_add(u, wtq[:, b_sl], t1)
                        nc.vector.tensor_sub(u, u, t2)
                        uq = uq2[:, b_sl]
                        nc.vector.tensor_scalar(
                            out=uq,
                            in0=u,
                            scalar1=0.0,
                            scalar2=65535.0,
                            op0=mybir.AluOpType.max,
                            op1=mybir.AluOpType.min,
                        )
                        nc.scalar.dma_start(
                            out=bass.AP(
                                Wout_s,
                                b * E * N + t * 128 * N + oc * OC,
                                [[N, 128], [1, OC]],
                            ).bitcast(u16),
                            in_=uq,
                        )

    _split_waits(nc)
    return nc


_cache = {}


def _get_nc():
    if "nc" not in _cache:
        nc = bass.Bass("TRN2", target_bir_lowering=False, debug=False)
        _emit(nc)
        _cache["nc"] = nc
    return _cache["nc"]


def kernel(Xd, Xpost, xbar_pre, xbar_post, W, dmap, A_p, A_d):
    Xd = np.asarray(Xd, dtype=np.float32)
    Xpost = np.asarray(Xpost, dtype=np.float32)
    xbar_pre = np.asarray(xbar_pre, dtype=np.float32)
    xbar_post = np.asarray(xbar_post, dtype=np.float32)
    W = np.asarray(W, dtype=np.float32)
    dmap = np.asarray(dmap, dtype=np.float32)
    A_p = np.asarray(A_p, dtype=np.float32)
    A_d = np.asarray(A_d, dtype=np.float32)

    # fold the per-column masks into the amplitudes (host-side, cheap)
    ApX = (A_p[None, :, :] * Xpost[:, None, :] * 65535.0).astype(ml_dtypes.bfloat16)
    AdX = (A_d[None, :, :] * xbar_post[:, None, :] * 65535.0).astype(ml_dtypes.bfloat16)
    dmap_bf = dmap.astype(ml_dtypes.bfloat16)
    Wq = (np.clip(W, 0.0, 1.0) * 65535.0 + 0.5).astype(np.uint16)

    in_maps = []
    for k in range(M):
        sl = slice(k * E, (k + 1) * E)
        # packed per-core setup constant: [ident | xps | xds] f32
        stp = np.zeros((128, 128 + 2 * NDB), dtype=np.float32)
        stp[:, 0:128] = np.eye(128, dtype=np.float32)
        xp = xbar_pre[:, :, sl].reshape(D, B, N_ETILES, 128)
        stp[:, 128 : 128 + NDB] = xp.transpose(3, 2, 0, 1).reshape(128, NDB)
        xd = Xd[:, :, sl].reshape(D, B, N_ETILES, 128)
        stp[:, 128 + NDB :] = xd.transpose(3, 2, 0, 1).reshape(128, NDB)
        in_maps.append(
            {
                "dmap_s": np.ascontiguousarray(dmap_bf[:, sl, :]),
                "ApX_s": np.ascontiguousarray(ApX[:, sl, :]),
                "AdX_s": np.ascontiguousarray(AdX[:, sl, :]),
                "W_s": np.ascontiguousarray(Wq[:, sl, :]).view(np.float16),
                "setup": stp,
            }
        )

    nc = _get_nc()
    res = run_bass_kernel_spmd(nc, in_maps, core_ids=list(range(M)))

    W_new = np.concatenate(
        [res.results[k]["Wout_s"].view(np.uint16) for k in range(M)], axis=1
    ).astype(np.float32) * np.float32(1.0 / 65535.0)
    # tiny trace updates on host (exact, <0.1% of the data volume)
    xbar_pre_new = ALPHA_P * xbar_pre + (1.0 - ALPHA_P) * Xd
    xbar_post_new = ALPHA_D * xbar_post + (1.0 - ALPHA_D) * Xpost
    W_prev = W
    return W_prev, W_new, xbar_pre_new, xbar_post_new


# revision 116
# speedup vs baseline: 1.0092x; 1.0021x over previous
"""Abbott STDP step kernel for 8 Trainium2 NeuronCores.

Math (per reference):
  dW_pot[b,e,o] = Xpost[b,o]   * (sum_d xbar_pre[d,b,e]*dmap[d,e,o]) * A_p[e,o]
  dW_dep[b,e,o] = xbar_post[b,o]* (sum_d Xd[d,b,e]      *dmap[d,e,o]) * A_d[e,o]
  W_new = clip(W + dW_pot - dW_dep, 0, 1)
  xbar_pre_new  = 0.95*xbar_pre  + 0.05*Xd      (host; trivially small)
  xbar_post_new = 0.90*xbar_post + 0.10*Xpost   (host; trivially small)

Sharding: presynaptic axis e (axis 1 of dmap/W/A, axis 2 of xbar_pre/Xd)
split into 8 slabs of 256; no cross-device reduction (d is the only
contracted axis).

Per-core compute:
  - d-contraction on the TensorEngine: 8 PSUM-accumulated matmuls per
    (b, pot/dep) output tile with diag(xbar[d,b,.]) stationary, in bf16
    (1 cycle/row; fp32 PSUM accumulation).  The diagonals are built once
    on DVE from a single packed host constant [ident | xps | xds].
  - The per-column masks and amplitudes are folded on the host into
    ApX[b]=A_p*Xpost[b]*65535, AdX[b]=A_d*xbar_post[b]*65535 (bf16), so
    the on-chip combine is just: t1=S_pot*ApX, t2=S_dep*AdX (DVE, PSUM
    source), u=W+t1-t2 (the add reads the u16 W tile directly; DVE
    converts inline, exactly), clip to [0,65535] (DVE fused max/min
    tensor_scalar casting to uint16).
  - W rides in 16-bit fixed point (W*65535 as uint16, exact to 7.6e-6
    since W_new is clipped to [0,1]): halves the W round-trip traffic.
    The u16 buffers wear a float16 facade at the DRAM/PJRT boundary
    (PJRT cannot bind u16); APs are bitcast inside the kernel.
  - dmap streams as quartered [128, 8*512] bf16 DMAs per (e-tile,
    o-chunk) on the SP HWDGE ring with 8-deep prefetch; W rides the
    same ring; ApX/AdX loads and W_new writes ride the ACT ring.
"""

import ml_dtypes
import numpy as np

import concourse.bass as bass
import concourse.mybir as mybir
from concourse.tile import TileContext
from concourse.bass_utils import run_bass_kernel_spmd

D, B, N = 8, 2, 2048
M = 8                  # cores
E = N // M             # 256 — per-core e-slab
OC = 512               # o chunk width
N_ETILES = E // 128    # 2
N_OCHUNKS = N // OC    # 4
ALPHA_P, ALPHA_D = 0.95, 0.9
WMAX = 1.0
NDB = N_ETILES * D * B  # 32 scalar columns per family

f32 = mybir.dt.float32
bf16 = mybir.dt.bfloat16
u16 = mybir.dt.uint16
f16 = mybir.dt.float16


def _split_waits(nc: bass.Bass):
    """Hoist all but one sem wait per instruction onto standalone
    same-engine InstNoOp carriers placed immediately before it.

    The walrus build used here rejects any instruction carrying more
    than one sync wait ("Too many sync wait commands"); engines execute
    in order, so a preceding same-engine wait is equivalent.
    """
    ctr = 0
    for f in nc.m.functions:
        for blk in f.blocks:
            il = blk.instructions
            i = 0
            while i < len(il):
                inst = il[i]
                si = inst.sync_info
                if si is not None and si.on_wait and len(si.on_wait) > 1:
                    waits = list(si.on_wait)
                    inst.sync_info = mybir.SyncInfo(
                        on_wait=[waits[0]], on_update=list(si.on_update or [])
                    )
                    for w in sorted(waits[1:], key=lambda x: -x.id):
                        nop = mybir.InstNoOp(name=f"wsplit-{ctr}", ins=[], outs=[])
                        ctr += 1
                        nop.engine = inst.engine
                        nop.sync_info = mybir.SyncInfo(on_wait=[w], on_update=[])
                        il.insert(i, nop)
                        i += 1
                i += 1


def _emit(nc: bass.Bass):
    dmap_s = nc.dram_tensor("dmap_s", [D, E, N], bf16, kind="ExternalInput")
    ApX_s = nc.dram_tensor("ApX_s", [B, E, N], bf16, kind="ExternalInput")
    AdX_s = nc.dram_tensor("AdX_s", [B, E, N], bf16, kind="ExternalInput")
    W_s = nc.dram_tensor("W_s", [B, E, N], f16, kind="ExternalInput")
    setup = nc.dram_tensor("setup", [128, 128 + 2 * NDB], f32, kind="ExternalInput")

    Wout_s = nc.dram_tensor("Wout_s", [B, E, N], f16, kind="ExternalOutput")

    with TileContext(nc) as tc:
        with (
            tc.tile_pool(name="persist", bufs=1) as persist,
            tc.tile_pool(name="stream", bufs=2) as stream,
            tc.tile_pool(name="temps", bufs=4) as temps,
            tc.tile_pool(name="outs", bufs=4) as outs,
            tc.tile_pool(name="psum", bufs=2, space="PSUM") as psum,
        ):
            # ---- one-time setup: one small DMA at the head of the SP ring ----
            setup_t = persist.tile([128, 128 + 2 * NDB], f32, name="setup_t")
            nc.sync.dma_start(out=setup_t, in_=setup[:, :])
            ident_t = setup_t[:, 0:128]

            def xp_col(t, d, b):
                c = 128 + t * 16 + d * 2 + b
                return setup_t[:, c : c + 1]

            def xd_col(t, d, b):
                c = 128 + NDB + t * 16 + d * 2 + b
                return setup_t[:, c : c + 1]

            # diag matrices diag(xbar_pre[d,b,e_tile]), diag(Xd[d,b,e_tile]),
            # in matmul-consumption order so the first group unblocks early
            dgp_tiles, dgd_tiles = {}, {}
            for t in range(N_ETILES):
                if t == 0:
                    # d-major blocks consume pot/dep alternating per (d, b):
                    # build in exactly that order
                    for d in range(D):
                        for b in range(B):
                            dp = persist.tile(
                                [128, 128], bf16, name=f"dgp_{t}_{d}_{b}"
                            )
                            nc.vector.tensor_scalar_mul(
                                dp, ident_t, xp_col(t, d, b)
                            )
                            dgp_tiles[(t, d, b)] = dp
                            dd = persist.tile(
                                [128, 128], bf16, name=f"dgd_{t}_{d}_{b}"
                            )
                            nc.vector.tensor_scalar_mul(
                                dd, ident_t, xd_col(t, d, b)
                            )
                            dgd_tiles[(t, d, b)] = dd
                else:
                    for b in range(B):
                        for d in range(D):
                            dp = persist.tile(
                                [128, 128], bf16, name=f"dgp_{t}_{d}_{b}"
                            )
                            nc.vector.tensor_scalar_mul(
                                dp, ident_t, xp_col(t, d, b)
                            )
                            dgp_tiles[(t, d, b)] = dp
                        for d in range(D):
                            dd = persist.tile(
                                [128, 128], bf16, name=f"dgd_{t}_{d}_{b}"
                            )
                            nc.vector.tensor_scalar_mul(
                                dd, ident_t, xd_col(t, d, b)
                            )
                            dgd_tiles[(t, d, b)] = dd

            # ---- main loop ----
            for t in range(N_ETILES):
                for oc in range(N_OCHUNKS):
                    # 8 delay slices as 4 quarter-DMAs into one [p, (d o)]
                    # tile — quarter granularity lets matmuls start as soon
                    # as their d-slices land (Tile tracks subtile deps)
                    dm_all = stream.tile([128, D * OC], bf16, name="dm_all", bufs=8)
                    h = D // 4
                    base = t * 128 * N + oc * OC
                    for q in range(4):
                        nc.sync.dma_start(
                            out=dm_all[:, q * h * OC : (q + 1) * h * OC],
                            in_=bass.AP(
                                dmap_s,
                                base + q * h * E * N,
                                [[N, 128], [E * N, h], [1, OC]],
                            ),
                        )
                    # one DMA per tensor covering both batches: [p, (b o)]
                    boff = t * 128 * N + oc * OC
                    bdims = [[N, 128], [E * N, B], [1, OC]]
                    wtq = stream.tile([128, B * OC], u16, name="wtq", bufs=3)
                    nc.sync.dma_start(out=wtq, in_=bass.AP(W_s, boff, bdims).bitcast(u16))
                    apx2 = stream.tile([128, B * OC], bf16, name="apx2", bufs=3)
                    nc.scalar.dma_start(out=apx2, in_=bass.AP(ApX_s, boff, bdims))
                    adx2 = stream.tile([128, B * OC], bf16, name="adx2", bufs=3)
                    nc.scalar.dma_start(out=adx2, in_=bass.AP(AdX_s, boff, bdims))
                    u2 = outs.tile([128, B * OC], f32, name="u2", bufs=3)
                    uq2 = outs.tile([128, B * OC], u16, name="uq2", bufs=3)

                    ps = {}
                    for b in range(B):
                        ps[("p", b)] = psum.tile([128, OC], f32, name=f"ps_pot{b}")
                        ps[("d", b)] = psum.tile([128, OC], f32, name=f"ps_dep{b}")
                    # d-major interleave on the first block only (PE would
                    # otherwise starve while iter-0 quarters stream in);
                    # group-major elsewhere to keep PSUM lifetimes short
                    if t == 0 and oc <= 1:
                        for d in range(D):
                            rhs = dm_all[:, d * OC : (d + 1) * OC]
                            for b in range(B):
                                nc.tensor.matmul(
                                    ps[("p", b)], dgp_tiles[(t, d, b)], rhs,
                                    start=(d == 0), stop=(d == D - 1),
                                )
                                nc.tensor.matmul(
                                    ps[("d", b)], dgd_tiles[(t, d, b)], rhs,
                                    start=(d == 0), stop=(d == D - 1),
                                )
                    else:
                        for b in range(B):
                            for key, tiles in (("p", dgp_tiles), ("d", dgd_tiles)):
                                for d in range(D):
                                    nc.tensor.matmul(
                                        ps[(key, b)], tiles[(t, d, b)],
                                        dm_all[:, d * OC : (d + 1) * OC],
                                        start=(d == 0), stop=(d == D - 1),
                                    )

                    for b in range(B):
                        ps_pot = ps[("p", b)]
                        ps_dep = ps[("d", b)]
                        b_sl = slice(b * OC, (b + 1) * OC)
                        t1 = temps.tile([128, OC], f32, name="t1")
                        nc.vector.tensor_mul(t1, ps_pot, apx2[:, b_sl])
                        t2 = temps.tile([128, OC], f32, name="t2")
                        nc.vector.tensor_mul(t2, ps_dep, adx2[:, b_sl])
                        u = u2[:, b_sl]
                        nc.vector.tensor_add(u, wtq[:, b_sl], t1)
                        nc.vector.tensor_sub(u, u, t2)
                        uq = uq2[:, b_sl]
                        nc.vector.tensor_scalar(
                            out=uq,
                            in0=u,
                            scalar1=0.0,
                            scalar2=65535.0,
                            op0=mybir.AluOpType.max,
                            op1=mybir.AluOpType.min,
                        )
                        nc.scalar.dma_start(
                            out=bass.AP(
                                Wout_s,
                                b * E * N + t * 128 * N + oc * OC,
                                [[N, 128], [1, OC]],
                            ).bitcast(u16),
                            in_=uq,
                        )

    _split_waits(nc)
    return nc


_cache = {}


def _get_nc():
    if "nc" not in _cache:
        nc = bass.Bass("TRN2", target_bir_lowering=False, debug=False)
        _emit(nc)
        _cache["nc"] = nc
    return _cache["nc"]


def kernel(Xd, Xpost, xbar_pre, xbar_post, W, dmap, A_p, A_d):
    Xd = np.asarray(Xd, dtype=np.float32)
    Xpost = np.asarray(Xpost, dtype=np.float32)
    xbar_pre = np.asarray(xbar_pre, dtype=np.float32)
    xbar_post = np.asarray(xbar_post, dtype=np.float32)
    W = np.asarray(W, dtype=np.float32)
    dmap = np.asarray(dmap, dtype=np.float32)
    A_p = np.asarray(A_p, dtype=np.float32)
    A_d = np.asarray(A_d, dtype=np.float32)

    # fold the per-column masks into the amplitudes (host-side, cheap)
    ApX = (A_p[None, :, :] * Xpost[:, None, :] * 65535.0).astype(ml_dtypes.bfloat16)
    AdX = (A_d[None, :, :] * xbar_post[:, None, :] * 65535.0).astype(ml_dtypes.bfloat16)
    dmap_bf = dmap.astype(ml_dtypes.bfloat16)
    Wq = (np.clip(W, 0.0, 1.0) * 65535.0 + 0.5).astype(np.uint16)

    in_maps = []
    for k in range(M):
        sl = slice(k * E, (k + 1) * E)
        # packed per-core setup constant: [ident | xps | xds] f32
        stp = np.zeros((128, 128 + 2 * NDB), dtype=np.float32)
        stp[:, 0:128] = np.eye(128, dtype=np.float32)
        xp = xbar_pre[:, :, sl].reshape(D, B, N_ETILES, 128)
        stp[:, 128 : 128 + NDB] = xp.transpose(3, 2, 0, 1).reshape(128, NDB)
        xd = Xd[:, :, sl].reshape(D, B, N_ETILES, 128)
        stp[:, 128 + NDB :] = xd.transpose(3, 2, 0, 1).reshape(128, NDB)
        in_maps.append(
            {
                "dmap_s": np.ascontiguousarray(dmap_bf[:, sl, :]),
                "ApX_s": np.ascontiguousarray(ApX[:, sl, :]),
                "AdX_s": np.ascontiguousarray(AdX[:, sl, :]),
                "W_s": np.ascontiguousarray(Wq[:, sl, :]).view(np.float16),
                "setup": stp,
            }
        )

    nc = _get_nc()
    res = run_bass_kernel_spmd(nc, in_maps, core_ids=list(range(M)))

    W_new = np.concatenate(
        [res.results[k]["Wout_s"].view(np.uint16) for k in range(M)], axis=1
    ).astype(np.float32) * np.float32(1.0 / 65535.0)
    # tiny trace updates on host (exact, <0.1% of the data volume)
    xbar_pre_new = ALPHA_P * xbar_pre + (1.0 - ALPHA_P) * Xd
    xbar_post_new = ALPHA_D * xbar_post + (1.0 - ALPHA_D) * Xpost
    W_prev = W
    return W_prev, W_new, xbar_pre_new, xbar_post_new
